# revision 3
# baseline (speedup 1.0000x reference)
"""Trainium2 Bass kernel for nn_GNOME_42588895707869 (GNN message passing + cdist).

v2: per-graph pipelined phase B.

Sharding: core k owns dst-nodes [1024k, 1024(k+1)) of BOTH graphs. The two
graphs are independent through all 6 GNN layers, so phase B alternates
g0-compute / g1-compute per layer and each graph's x-AllGather + overflow
dma_gather overlaps the other graph's compute.

x rows of graph g are AllGathered in layout [128, 8*128] per rank
(partition = src%128, cols = window-in-rank * H) so the X_sb reload is
128 x 4KB contiguous descriptors. One-hot gather/scatter matrices live in
SBUF as fp8 for the whole phase (loaded once); only the edge-MLP output e
streams per layer. The segment-sum scatter runs with the message as the
stationary operand, accumulating agg^T (H x dst) directly in PSUM, so the
node MLP consumes it without transposes.
"""
import sys

sys.path.insert(0, "/opt/trn_rl_repo")

import numpy as np  # noqa: E402

N = 8192
H = 128
L = 6
CAT = 768
E = 131072
NQ = 2048          # local nodes per core (1024 per graph)
NS = 1024          # nodes per graph per core
WSZ = 128
C1 = 256           # class-1 chunks (2 graphs x 8 wd x 16 a)
XIN = 64           # features(48) + RW(16)
ROWS_D = 1024      # cdist rows per core
EPS = 1e-12
MR = CAT + 2       # m rows (row 0 = nsq hi, row 1 = nsq lo)


# ---------------------------------------------------------------- host prep
def _pack_all(ei1, ei2, ef1, ef2):
    """Bucket edges per core/graph into (wd, src-window) cells + overflow.

    Overflow edges are packed densely per graph (not per wd): each 128-slot
    residual chunk spans dst windows and is scattered with 8 per-wd matmuls.
    """
    cores = []
    for k in range(8):
        graphs = []
        for g, (ei, ef) in enumerate(((ei1, ef1), (ei2, ef2))):
            src = np.asarray(ei[0]).astype(np.int64)
            dst = np.asarray(ei[1]).astype(np.int64)
            sel = (dst // NS) == k
            s_k, d_k = src[sel], dst[sel] - k * NS
            ef_k = np.asarray(ef, np.float32)[sel]
            cells = [[[] for _ in range(64)] for _ in range(8)]
            over = []
            wd = d_k // WSZ
            drel = d_k % WSZ
            W = s_k // WSZ
            p = s_k % WSZ
            for i in range(len(s_k)):
                cell = cells[wd[i]][W[i]]
                if len(cell) < 32:
                    cell.append((p[i], drel[i], ef_k[i]))
                else:
                    over.append((s_k[i], d_k[i], ef_k[i]))
            graphs.append((cells, over))
        cores.append(graphs)
    owc = max((len(over) + 127) // 128
              for graphs in cores for cells, over in graphs)
    owc = max(owc, 1)
    return cores, owc


def _core_layout(graphs, owc):
    import ml_dtypes
    C = (C1 + 2 * owc + 15) // 16 * 16   # e/ef chunks, padded to slab size
    oh = np.zeros((128, C1 * 128), dtype=ml_dtypes.float8_e4m3fn)
    ohd = np.zeros((128, C1 * 128), dtype=ml_dtypes.float8_e4m3fn)
    ohr = np.zeros((128, 2 * owc * NS), dtype=ml_dtypes.float8_e4m3fn)
    srco = np.zeros(2 * owc * 128, dtype=np.int16)
    ef_perm = np.zeros((C * 128, 9), dtype=ml_dtypes.bfloat16)
    for g, (cells, over) in enumerate(graphs):
        for wd in range(8):
            for a in range(16):
                c = g * 128 + wd * 16 + a
                base = c * 128
                for q in range(4):
                    cell = cells[wd][4 * a + q]
                    for e, (p, drel, ef) in enumerate(cell):
                        s = q * 32 + e
                        oh[int(p), base + s] = 1.0
                        ohd[s, base + int(drel)] = 1.0
                        ef_perm[base + s, :8] = ef
                        ef_perm[base + s, 8] = 1.0
        for ro in range(owc):
            c = C1 + g * owc + ro
            base = c * 128
            seg = over[ro * 128:(ro + 1) * 128]
            sbase = (g * owc + ro) * 128
            for e, (s_glob, d_glob, ef) in enumerate(seg):
                # block index into x_rows_g viewed as [8192, 128]
                srco[sbase + e] = (8 * (128 * (s_glob // NS) + s_glob % 128)
                                  + (s_glob % NS) // 128)
                ohr[e, (g * owc + ro) * NS + int(d_glob)] = 1.0
                ef_perm[base + e, :8] = ef
                ef_perm[base + e, 8] = 1.0
    return oh, ohd, ohr, srco, np.ascontiguousarray(ef_perm.T)


def _idx_sb(idx):
    n = idx.shape[0]
    assert n % 16 == 0
    a = np.ascontiguousarray(idx.astype(np.int16).reshape(n // 16, 16).T)
    return np.tile(a, (8, 1)).copy()


# ---------------------------------------------------------------- program
_prog_cache = {}


def _build_program(OWC):
    import concourse.bass as bass  # noqa: F401
    import concourse.mybir as mybir
    from concourse import bacc
    from concourse.tile import TileContext
    from concourse.masks import make_identity

    f32 = mybir.dt.float32
    f32r = mybir.dt.float32r
    bf16 = mybir.dt.bfloat16
    f8 = mybir.dt.float8e4
    i16 = mybir.dt.int16
    AF = mybir.ActivationFunctionType
    Alu = mybir.AluOpType

    ES = 16                        # phase-A edge-MLP slab
    C = (C1 + 2 * OWC + 15) // 16 * 16   # e/ef chunks, padded to slab size

    nc = bacc.Bacc("TRN2", num_devices=8, num_swdge_queues=4)

    xin = nc.declare_dram_parameter("xin", [XIN + 1, NQ], f32, isOutput=False)
    wpre = nc.declare_dram_parameter("wpre", [XIN + 1, H], f32, isOutput=False)
    wedge = nc.declare_dram_parameter("wedge", [9, H], f32, isOutput=False)
    efT = nc.declare_dram_parameter("efT", [9, C * 128], bf16, isOutput=False)
    oh_d = nc.declare_dram_parameter("oh_d", [128, C1 * 128], f8,
                                     isOutput=False)
    ohd_d = nc.declare_dram_parameter("ohd_d", [128, C1 * 128], f8,
                                      isOutput=False)
    ohr_d = nc.declare_dram_parameter("ohr_d", [128, 2 * OWC * NS], f8,
                                      isOutput=False)
    srcov = nc.declare_dram_parameter("srcov", [128, 2 * OWC * 8], i16,
                                      isOutput=False)
    gw1 = nc.declare_dram_parameter("gw1", [L, H, H], f32, isOutput=False)
    gw2 = nc.declare_dram_parameter("gw2", [L, H, H], f32, isOutput=False)
    gb1t = nc.declare_dram_parameter("gb1t", [H, L], f32, isOutput=False)
    gb2t = nc.declare_dram_parameter("gb2t", [H, L], f32, isOutput=False)
    wo1 = nc.declare_dram_parameter("wo1", [CAT, 2 * CAT], f32, isOutput=False)
    wo2 = nc.declare_dram_parameter("wo2", [2 * CAT, CAT], f32, isOutput=False)
    bo1t = nc.declare_dram_parameter("bo1t", [H, 12], f32, isOutput=False)
    bo2t = nc.declare_dram_parameter("bo2t", [H, 6], f32, isOutput=False)
    out = nc.declare_dram_parameter("out", [ROWS_D, N], f32, isOutput=True)

    x_rows = [nc.dram_tensor(f"x_rows{g}", [8 * 128, 8 * H], bf16,
                             addr_space="Shared") for g in range(2)]
    x_ag_in = [nc.dram_tensor(f"x_ag_in{g}", [128, 8, H], bf16)
               for g in range(2)]
    e_hbm = nc.dram_tensor("e_hbm", [128, C * H], bf16)
    outs_hbm = nc.dram_tensor("outs_hbm", [L, H, NQ], bf16)
    m_loc = nc.dram_tensor("m_loc", [MR, NS], bf16)
    mag_in = nc.dram_tensor("mag_in", [MR, NS], bf16)
    mT_all = nc.dram_tensor("mT_all", [8 * MR, NS], bf16, addr_space="Shared")

    allg = [[0, 1, 2, 3, 4, 5, 6, 7]]

    with TileContext(nc) as tc:
        cpool = tc.alloc_tile_pool(name="const", bufs=1)
        ident = cpool.tile([128, 128], f32)
        make_identity(nc, ident[:])
        identr = cpool.tile([128, 128], f32r)
        nc.vector.tensor_copy(identr[:], ident[:])
        identb = cpool.tile([128, 128], bf16)
        nc.vector.tensor_copy(identb[:], ident[:])
        gb1s = cpool.tile([H, L], f32)
        nc.sync.dma_start(out=gb1s[:], in_=gb1t[:])
        gb2s = cpool.tile([H, L], f32)
        nc.sync.dma_start(out=gb2s[:], in_=gb2t[:])
        w1r = cpool.tile([H, L, H], f32r)
        w2r = cpool.tile([H, L, H], f32r)
        xcur = cpool.tile([H, NQ], f32r)
        feat_t = cpool.tile([H, NQ], f32r)
        zc = cpool.tile([1, 128], bf16, name="zc")
        nc.vector.memset(zc[:], 0.0)
        zr = cpool.tile([1, 512], bf16, name="zr")
        nc.vector.memset(zr[:], 0.0)

        gpool = tc.alloc_tile_pool(name="grid", bufs=1)
        oh_sb = gpool.tile([128, C1, 128], f8)
        ohd_sb = gpool.tile([128, C1, 128], f8)
        ohr_sb = gpool.tile([128, 2 * OWC, NS], f8)
        srcot = gpool.tile([128, 2 * OWC * 8], i16)
        X_sb = [gpool.tile([128, 8, 8, H], bf16, name=f"X_sb{g}")
                for g in range(2)]

        def load_grid():
            # issued after the phase-A exports so xin/weight DMAs go first
            nc.sync.dma_start(
                out=oh_sb[:], in_=oh_d[:].rearrange("p (a m) -> p a m", m=128))
            nc.sync.dma_start(
                out=ohd_sb[:],
                in_=ohd_d[:].rearrange("p (a m) -> p a m", m=128))
            nc.sync.dma_start(
                out=ohr_sb[:],
                in_=ohr_d[:].rearrange("p (a m) -> p a m", m=NS))
            nc.sync.dma_start(out=srcot[:], in_=srcov[:])

        def export_x(pool, get_pst, g):
            """Transpose xcur cols of graph g -> x_ag_in[g], AllGather."""
            xr2 = pool.tile([128, 8, H], bf16, tag=f"xr2_{g}")
            for t in range(8):
                pst = get_pst(t)
                col = g * NS + t * 128
                nc.tensor.transpose(out=pst,
                                    in_=xcur[:, col:col + 128],
                                    identity=identr[:])
                nc.scalar.activation(xr2[:, t, :], pst, AF.Copy)
            nc.sync.dma_start(out=x_ag_in[g][:, 0:4, :], in_=xr2[:, 0:4, :])
            nc.sync.dma_start(out=x_ag_in[g][:, 4:8, :], in_=xr2[:, 4:8, :])
            nc.gpsimd.collective_compute(
                "AllGather", Alu.bypass, ins=[x_ag_in[g][:]],
                outs=[x_rows[g][:]], replica_groups=allg)

        def load_X(g):
            # issued on gpsimd: sits right behind the AllGather there, so the
            # wait doesn't head-of-line-block the sync queue's slab loads
            nc.gpsimd.dma_start(
                out=X_sb[g][:],
                in_=x_rows[g][:].rearrange("(k p) (t m) -> p k t m",
                                           p=128, m=128))

        # ---------------- phase A ------------------------------------
        with tc.tile_pool(name="phA", bufs=2) as pa, \
             tc.tile_pool(name="psA", bufs=2, space="PSUM") as ppa:
            wtmp = pa.tile([H, L, H], f32, tag="wtmp", bufs=1)
            nc.sync.dma_start(out=wtmp[:], in_=gw1[:].rearrange("l k m -> k l m"))
            nc.vector.tensor_copy(w1r[:], wtmp[:])
            wtmp2 = pa.tile([H, L, H], f32, tag="wtmp2", bufs=1)
            nc.sync.dma_start(out=wtmp2[:], in_=gw2[:].rearrange("l k m -> k l m"))
            nc.vector.tensor_copy(w2r[:], wtmp2[:])

            xinf = pa.tile([XIN + 1, NQ], f32, tag="xinf", bufs=1)
            for qd in range(4):
                nc.sync.dma_start(out=xinf[:, qd * 512:(qd + 1) * 512],
                                  in_=xin[:, qd * 512:(qd + 1) * 512])
            xinr = pa.tile([XIN + 1, NQ], f32r, tag="xinr", bufs=1)
            nc.vector.tensor_copy(xinr[:], xinf[:])
            wpref = pa.tile([XIN + 1, H], f32, tag="wpref", bufs=1)
            nc.sync.dma_start(out=wpref[:], in_=wpre[:])
            wprer = pa.tile([XIN + 1, H], f32r, tag="wprer", bufs=1)
            nc.vector.tensor_copy(wprer[:], wpref[:])
            for nt in range(NQ // 512):
                ps = ppa.tile([H, 512], f32, space="PSUM", tag="psx")
                nc.tensor.matmul(ps[:], lhsT=wprer[:],
                                 rhs=xinr[:, nt * 512:(nt + 1) * 512],
                                 start=True, stop=True)
                nc.vector.tensor_copy(xcur[:, nt * 512:(nt + 1) * 512], ps[:])
            nc.vector.tensor_copy(feat_t[:], xcur[:])

            def pst_a(t):
                pst_t = ppa.tile([128, 128], f32r, space="PSUM", tag="psT")
                return pst_t[:]
            export_x(pa, pst_a, 0)
            export_x(pa, pst_a, 1)
            load_grid()

            # edge MLP -> e_hbm bf16 (streamed back per layer)
            weg = pa.tile([9, H], f32, tag="weg", bufs=1)
            nc.sync.dma_start(out=weg[:], in_=wedge[:])
            wegb = pa.tile([9, H], bf16, tag="wegb", bufs=1)
            nc.vector.tensor_copy(wegb[:], weg[:])
            for sl in range(C // ES):
                c0 = sl * ES
                eslabb = pa.tile([9, ES * 128], bf16, tag="eslabb")
                nc.sync.dma_start(out=eslabb[:],
                                  in_=efT[:, c0 * 128:(c0 + ES) * 128])
                egs = pa.tile([128, ES, H], bf16, tag="egs")
                for c4 in range(ES // 4):
                    pse = ppa.tile([128, 4, H], f32, space="PSUM", tag="pse")
                    for ci in range(4):
                        cc = c4 * 4 + ci
                        nc.tensor.matmul(
                            pse[:, ci, :],
                            lhsT=eslabb[:, cc * 128:(cc + 1) * 128],
                            rhs=wegb[:], start=True, stop=True)
                    if c4 % 2 == 0:
                        nc.scalar.activation(egs[:, c4 * 4:c4 * 4 + 4, :],
                                             pse[:], AF.Copy)
                    else:
                        nc.vector.tensor_copy(egs[:, c4 * 4:c4 * 4 + 4, :],
                                              pse[:])
                nc.sync.dma_start(
                    out=e_hbm[:, c0 * H:(c0 + ES) * H], in_=egs[:])
            load_X(0)
            load_X(1)

        # ---------------- phase B: 6 GNN layers ----------------------
        with tc.tile_pool(name="phB", bufs=2) as pb, \
             tc.tile_pool(name="mgB", bufs=3) as pmg, \
             tc.tile_pool(name="srB", bufs=3) as psr, \
             tc.tile_pool(name="agB", bufs=1, space="PSUM") as pag, \
             tc.tile_pool(name="psM", bufs=2, space="PSUM") as ppm:
            aggT = [pag.tile([128, 8, 128], f32, space="PSUM",
                             tag=f"aggT{g}", name=f"aggT{g}")
                    for g in range(2)]
            pworks = [pag.tile([128, 4, 128], f32, space="PSUM",
                               tag=f"pwork{i}", name=f"pwork{i}")
                      for i in range(2)]

            def pst_b(t):
                return pworks[1][:, t % 4, :].bitcast(f32r)

            def gather_og(g):
                og = pb.tile([128, OWC, H], bf16, tag="og", bufs=3)
                xsrc = x_rows[g][:].rearrange("a (b m) -> (a b) m", m=128)
                i0 = g * OWC * 8
                qn = 0
                for c0_ in range(0, OWC, max(1, (OWC + 3) // 4)):
                    c1_ = min(OWC, c0_ + max(1, (OWC + 3) // 4))
                    n_ = (c1_ - c0_) * 128
                    nc.gpsimd.dma_gather(
                        og[:, c0_:c1_, :], xsrc,
                        srcot[:, i0 + c0_ * 8:i0 + c1_ * 8],
                        n_, n_, H, elem_step=H,
                        single_packet=False, queue_num=qn)
                    qn = (qn + 1) % 4
                return og

            def prepare_resid(g, og):
                """Residual messages relu(x[src] + e) for graph g's overflow.
                Emitted at the tail of the section before the consumer, when
                the og drain has long finished."""
                eov = pb.tile([128, OWC, H], bf16, tag="eov", bufs=3)
                co0 = C1 + g * OWC
                nc.sync.dma_start(
                    out=eov[:],
                    in_=e_hbm[:, co0 * H:(co0 + OWC) * H].rearrange(
                        "p (a m) -> p a m", m=H))
                nc.vector.tensor_tensor(out=og[:], in0=og[:],
                                        in1=eov[:], op=Alu.add)
                msgo = pb.tile([128, OWC, H], bf16, tag="msgo", bufs=3)
                nc.scalar.activation(msgo[:], og[:], AF.Relu)
                return msgo

            # layer-0 overflow gathers run during phase A's edge MLP
            ogs = [gather_og(0), gather_og(1)]
            msgos = [prepare_resid(0, ogs[0]), prepare_resid(1, ogs[1])]
            pending_prep = None
            for l in range(L):
                for g in range(2):
                    # zero the agg accumulators
                    nc.tensor.matmul(aggT[g][:, 0:4, :], lhsT=zc[:], rhs=zr[:],
                                     start=True, stop=False)
                    nc.tensor.matmul(aggT[g][:, 4:8, :], lhsT=zc[:], rhs=zr[:],
                                     start=True, stop=False)

                    # --- residual scatters first: msgo was prepared at the
                    # previous section's tail, so it is ready
                    msgo = msgos[g]
                    for ro in range(OWC):
                        for wd in range(8):
                            nc.tensor.matmul(
                                aggT[g][:, wd, :],
                                lhsT=msgo[:, ro, :],
                                rhs=ohr_sb[:, g * OWC + ro,
                                           wd * 128:(wd + 1) * 128],
                                start=False, stop=False)

                    # scatter MMs trail their group by one, so the relu of
                    # group i overlaps the identity+gather MMs of group i+1
                    pending = None
                    for wd in range(8):
                        c0 = g * 128 + wd * 16
                        esl = psr.tile([128, 16, H], bf16, tag="esl")
                        nc.sync.dma_start(
                            out=esl[:],
                            in_=e_hbm[:, c0 * H:(c0 + 16) * H].rearrange(
                                "p (a m) -> p a m", m=H))
                        for jj in range(0, 16, 4):
                            grp = wd * 4 + jj // 4
                            pwork = pworks[grp % 2]
                            nc.tensor.matmul(pwork[:], lhsT=identb[:],
                                             rhs=esl[:, jj:jj + 4, :],
                                             start=True, stop=False)
                            for ci in range(4):
                                a = jj + ci
                                c = c0 + a
                                for q in range(4):
                                    W = 4 * a + q
                                    nc.tensor.matmul(
                                        pwork[32 * q:32 * q + 32, ci, :],
                                        lhsT=oh_sb[:, c, 32 * q:32 * q + 32],
                                        rhs=X_sb[g][:, W // 8, W % 8, :],
                                        start=False, stop=(q == 3),
                                        tile_position=(0, 32 * q))
                            if pending is not None:
                                pm, pwd, pc0, plast = pending
                                for ci in range(4):
                                    nc.tensor.matmul(
                                        aggT[g][:, pwd, :],
                                        lhsT=pm[:, ci, :],
                                        rhs=ohd_sb[:, pc0 + ci, :],
                                        start=False,
                                        stop=(plast and ci == 3))
                            msg4 = pmg.tile([128, 4, 128], bf16, tag="msg")
                            if grp % 2 == 0:
                                nc.scalar.activation(msg4[:], pwork[:], AF.Relu)
                            else:
                                nc.vector.tensor_relu(msg4[:], pwork[:])
                            pending = (msg4, wd, c0 + jj, jj == 12)
                    pm, pwd, pc0, plast = pending
                    for ci in range(4):
                        nc.tensor.matmul(aggT[g][:, pwd, :],
                                         lhsT=pm[:, ci, :],
                                         rhs=ohd_sb[:, pc0 + ci, :],
                                         start=False, stop=(plast and ci == 3))

                    # --- node MLP (graph g's 1024 nodes)
                    for nt in range(2):
                        col = g * NS + nt * 512
                        sl_ = slice(col, col + 512)
                        ht = pb.tile([H, 512], f32r, tag="ht")
                        nc.vector.scalar_tensor_tensor(
                            out=ht[:], in0=aggT[g][:, 4 * nt:4 * nt + 4, :],
                            scalar=0.0, in1=xcur[:, sl_],
                            op0=Alu.bypass, op1=Alu.add)
                        ps1 = ppm.tile([H, 512], f32, space="PSUM", tag="psmlp")
                        nc.tensor.matmul(ps1[:], lhsT=w1r[:, l, :], rhs=ht[:],
                                         start=True, stop=True)
                        t1 = pb.tile([H, 512], f32r, tag="t1")
                        nc.scalar.activation(t1[:], ps1[:], AF.Relu,
                                             bias=gb1s[:, l:l + 1])
                        ps2 = ppm.tile([H, 512], f32, space="PSUM", tag="psmlp")
                        nc.tensor.matmul(ps2[:], lhsT=w2r[:, l, :], rhs=t1[:],
                                         start=True, stop=True)
                        if l in (1, 3):
                            s0 = pb.tile([H, 512], f32, space="SBUF", tag="s0")
                            nc.scalar.activation(s0[:], ps2[:], AF.Identity,
                                                 bias=gb2s[:, l:l + 1])
                            nc.vector.tensor_tensor(out=feat_t[:, sl_],
                                                    in0=s0[:],
                                                    in1=feat_t[:, sl_],
                                                    op=Alu.add)
                            nc.vector.tensor_relu(xcur[:, sl_], feat_t[:, sl_])
                        else:
                            nc.scalar.activation(xcur[:, sl_], ps2[:], AF.Relu,
                                                 bias=gb2s[:, l:l + 1])
                    xob = pb.tile([H, NS], bf16, tag="xob")
                    nc.vector.tensor_copy(xob[:], xcur[:, g * NS:(g + 1) * NS])
                    nc.sync.dma_start(
                        out=outs_hbm[l][:, g * NS:(g + 1) * NS], in_=xob[:])
                    # prepare the next section's residual messages BEFORE
                    # emitting this section's gather, so the coarse
                    # cross-engine wait doesn't bind the prep to it
                    if pending_prep is not None and not (l == L - 1 and g == 1):
                        msgos[pending_prep] = prepare_resid(
                            pending_prep, ogs[pending_prep])
                    pending_prep = g
                    if l < L - 1:
                        export_x(pb, pst_b, g)
                        load_X(g)
                        ogs[g] = gather_og(g)

        gpool.release()

        # ---------------- phase C: output MLP ------------------------
        with tc.tile_pool(name="phCw", bufs=1) as pcw, \
             tc.tile_pool(name="phC", bufs=2) as pc, \
             tc.tile_pool(name="phCh", bufs=1) as pch, \
             tc.tile_pool(name="psC", bufs=4, space="PSUM") as ppc:
            wo1r = pcw.tile([128, 6, 2 * CAT], bf16, tag="wo1r")
            wo2r = pcw.tile([128, 12, CAT], bf16, tag="wo2r")
            for kc in range(6):
                wt = pc.tile([128, 2 * CAT], f32, tag="wldtmp")
                nc.sync.dma_start(
                    out=wt[:],
                    in_=wo1[:].rearrange("(a p) m -> a p m", p=128)[kc])
                nc.vector.tensor_copy(wo1r[:, kc, :], wt[:])
            for kc in range(12):
                wt = pc.tile([128, CAT], f32, tag="wldtmp")
                nc.sync.dma_start(
                    out=wt[:],
                    in_=wo2[:].rearrange("(a p) m -> a p m", p=128)[kc])
                nc.vector.tensor_copy(wo2r[:, kc, :], wt[:])
            bo1s = pcw.tile([H, 12], f32, tag="bo1s")
            nc.sync.dma_start(out=bo1s[:], in_=bo1t[:])
            bo2s = pcw.tile([H, 6], f32, tag="bo2s")
            nc.sync.dma_start(out=bo2s[:], in_=bo2t[:])
            ones_r = pcw.tile([128, 1], f32r, tag="ones_r")
            onesf = pcw.tile([128, 1], f32, tag="onesf")
            nc.vector.memset(onesf[:], 1.0)
            nc.vector.tensor_copy(ones_r[:], onesf[:])

            nsq_sb = pcw.tile([1, NQ], f32, tag="nsq_sb")
            for nt in (2, 3, 0, 1):
                sl_ = slice(nt * 512, (nt + 1) * 512)
                ne_t = []
                for kc in range(6):
                    nt_t = pc.tile([H, 512], bf16, tag=f"ne{kc}")
                    nc.sync.dma_start(out=nt_t[:], in_=outs_hbm[kc][:, sl_])
                    ne_t.append(nt_t)
                h1 = pch.tile([128, 12, 512], bf16, tag="h1")
                for mt in range(12):
                    ps = ppc.tile([128, 512], f32, space="PSUM", tag="psc")
                    for kc in range(6):
                        nc.tensor.matmul(
                            ps[:], lhsT=wo1r[:, kc, mt * 128:(mt + 1) * 128],
                            rhs=ne_t[kc][:], start=(kc == 0), stop=(kc == 5))
                    nc.scalar.activation(h1[:, mt, :], ps[:], AF.Relu,
                                         bias=bo1s[:, mt:mt + 1])
                sqsum = ppc.tile([1, 512], f32, space="PSUM", tag="sqsum")
                # local cols 0-1023 -> m_loc, 1024-2047 -> mag_in
                mdst = m_loc if nt < 2 else mag_in
                coff = nt * 512 if nt < 2 else (nt - 2) * 512
                for m2 in range(6):
                    ps = ppc.tile([128, 512], f32, space="PSUM", tag="psc")
                    for kc in range(12):
                        nc.tensor.matmul(
                            ps[:], lhsT=wo2r[:, kc, m2 * 128:(m2 + 1) * 128],
                            rhs=h1[:, kc, :], start=(kc == 0), stop=(kc == 11))
                    mtile = pc.tile([128, 512], f32, tag="mtile")
                    nc.scalar.activation(mtile[:], ps[:], AF.Identity,
                                         bias=bo2s[:, m2:m2 + 1])
                    mtileb = pc.tile([128, 512], bf16, tag="mtileb")
                    nc.vector.tensor_copy(mtileb[:], mtile[:])
                    nc.sync.dma_start(
                        out=mdst[2 + m2 * 128:2 + (m2 + 1) * 128,
                                 coff:coff + 512],
                        in_=mtileb[:])
                    sq = pc.tile([128, 512], f32r, tag="sq")
                    nc.vector.tensor_tensor(out=sq[:], in0=mtileb[:],
                                            in1=mtileb[:], op=Alu.mult)
                    nc.tensor.matmul(sqsum[:], lhsT=ones_r[:], rhs=sq[:],
                                     start=(m2 == 0), stop=(m2 == 5))
                nc.vector.tensor_copy(nsq_sb[:, sl_], sqsum[:])
                if nt == 3:
                    # graph-2 half done: finalize its norms and launch the
                    # AllGather now so it overlaps the graph-1 half + more
                    nsqb = pcw.tile([1, NQ], bf16, tag="nsqb")
                    nlo = pcw.tile([1, NQ], f32, tag="nlo")
                    nlob = pcw.tile([1, NQ], bf16, tag="nlob")
                    nc.vector.tensor_copy(nsqb[:, NS:NQ], nsq_sb[:, NS:NQ])
                    nc.vector.tensor_tensor(out=nlo[:, NS:NQ],
                                            in0=nsq_sb[:, NS:NQ],
                                            in1=nsqb[:, NS:NQ],
                                            op=Alu.subtract)
                    nc.vector.tensor_copy(nlob[:, NS:NQ], nlo[:, NS:NQ])
                    nc.sync.dma_start(out=mag_in[0:1, :], in_=nsqb[:, NS:NQ])
                    nc.sync.dma_start(out=mag_in[1:2, :], in_=nlob[:, NS:NQ])
                    nc.gpsimd.collective_compute(
                        "AllGather", Alu.bypass, ins=[mag_in[:]],
                        outs=[mT_all[:]], replica_groups=allg)
            nc.vector.tensor_copy(nsqb[:, 0:NS], nsq_sb[:, 0:NS])
            nc.vector.tensor_tensor(out=nlo[:, 0:NS], in0=nsq_sb[:, 0:NS],
                                    in1=nsqb[:, 0:NS], op=Alu.subtract)
            nc.vector.tensor_copy(nlob[:, 0:NS], nlo[:, 0:NS])
            nc.sync.dma_start(out=m_loc[0:1, :], in_=nsqb[:, 0:NS])
            nc.sync.dma_start(out=m_loc[1:2, :], in_=nlob[:, 0:NS])

        # ---------------- phase D: cdist -----------------------------
        with tc.tile_pool(name="phD1", bufs=1) as pd1, \
             tc.tile_pool(name="phD", bufs=2) as pd, \
             tc.tile_pool(name="ohD", bufs=3) as pdd, \
             tc.tile_pool(name="psD", bufs=4, space="PSUM") as ppd:
            onesrow = pd1.tile([128, 128], bf16, tag="onesrow")
            nc.vector.memset(onesrow[:], 0.0)
            nc.vector.memset(onesrow[0:2, :], 1.0)
            m1pre = pd1.tile([128, 6, NS], bf16, tag="m1pre")
            nc.sync.dma_start(
                out=m1pre[:],
                in_=m_loc[2:2 + CAT, :].rearrange("(a p) m -> p a m", p=128))
            m1r = pd1.tile([128, 6, NS], bf16, tag="m1r")
            nc.vector.tensor_scalar_mul(m1r[:], m1pre[:], -2.0)
            n1hi = pd1.tile([128, 8], bf16, tag="n1hi")
            nc.sync.dma_start(
                out=n1hi[:],
                in_=m_loc[0:1, :].rearrange("o (b p) -> (o p) b", p=128))
            n1lo = pd1.tile([128, 8], bf16, tag="n1lo")
            nc.sync.dma_start(
                out=n1lo[:],
                in_=m_loc[1:2, :].rearrange("o (b p) -> (o p) b", p=128))
            n1f = pd1.tile([128, 8], f32, tag="n1f")
            nc.vector.tensor_tensor(out=n1f[:], in0=n1hi[:], in1=n1lo[:],
                                    op=Alu.add)
            epsf = pd1.tile([128, 8], f32, tag="epsf")
            nc.vector.tensor_scalar(out=epsf[:], in0=n1f[:], scalar1=-1.0,
                                    scalar2=EPS, op0=Alu.mult, op1=Alu.add)

            for s in range(16):
                rb, soff = s // 2, (s % 2) * 512
                base = rb * MR
                st_r = pd.tile([128, 6, 512], bf16, tag="st_r")
                nc.sync.dma_start(
                    out=st_r[:],
                    in_=mT_all[base + 2:base + 2 + CAT,
                               soff:soff + 512].rearrange(
                        "(a p) m -> p a m", p=128))
                st6 = pd.tile([128, 512], bf16, tag="st6")
                nc.sync.dma_start(out=st6[:],
                                  in_=mT_all[base:base + 128,
                                             soff:soff + 512])
                for b in range(8):
                    psd = ppd.tile([128, 512], f32, space="PSUM", tag="psd")
                    for kc in range(6):
                        nc.tensor.matmul(psd[:],
                                         lhsT=m1r[:, kc, b * 128:(b + 1) * 128],
                                         rhs=st_r[:, kc, :],
                                         start=(kc == 0), stop=False)
                    nc.tensor.matmul(psd[:], lhsT=onesrow[:], rhs=st6[:],
                                     start=False, stop=True)
                    s1 = pdd.tile([128, 512], f32, tag="s1")
                    nc.vector.tensor_scalar(out=s1[:], in0=psd[:],
                                            scalar1=epsf[:, b:b + 1],
                                            scalar2=0.0,
                                            op0=Alu.max, op1=Alu.add)
                    dt_ = pdd.tile([128, 512], f32, tag="dt_")
                    nc.scalar.activation(dt_[:], s1[:], AF.Sqrt,
                                         bias=n1f[:, b:b + 1])
                    nc.sync.dma_start(
                        out=out[b * 128:(b + 1) * 128, s * 512:(s + 1) * 512],
                        in_=dt_[:])
        cpool.release()

    nc.compile()
    return nc


# ---------------------------------------------------------------- entry
def kernel(**inputs):
    from concourse.bass_utils import run_bass_kernel_spmd

    cores, owc = _pack_all(inputs["edge_index_1"], inputs["edge_index_2"],
                           inputs["e_features1"], inputs["e_features2"])

    feats = [np.asarray(inputs["features_1"], dtype=np.float32),
             np.asarray(inputs["features_2"], dtype=np.float32)]
    rws = [np.asarray(inputs["RW_1"], dtype=np.float32),
           np.asarray(inputs["RW_2"], dtype=np.float32)]

    wpre_aug = np.vstack([np.asarray(inputs["W_pre"], dtype=np.float32),
                          np.asarray(inputs["b_pre"], dtype=np.float32)[None]])
    wedge_aug = np.vstack([np.asarray(inputs["W_edge"], dtype=np.float32),
                           np.asarray(inputs["b_edge"], dtype=np.float32)[None]])
    gw1 = np.asarray(inputs["gnn_w1"], dtype=np.float32)
    gw2 = np.asarray(inputs["gnn_w2"], dtype=np.float32)
    gb1t = np.ascontiguousarray(np.asarray(inputs["gnn_b1"], np.float32).T)
    gb2t = np.ascontiguousarray(np.asarray(inputs["gnn_b2"], np.float32).T)
    wo1 = np.asarray(inputs["W_out1"], dtype=np.float32)
    wo2 = np.asarray(inputs["W_out2"], dtype=np.float32)
    bo1t = np.ascontiguousarray(
        np.asarray(inputs["b_out1"], np.float32).reshape(12, 128).T)
    bo2t = np.ascontiguousarray(
        np.asarray(inputs["b_out2"], np.float32).reshape(6, 128).T)

    in_maps = []
    for k in range(8):
        oh, ohd, ohr, srco, efT = _core_layout(cores[k], owc)
        xg1 = np.concatenate(
            [feats[0][k * NS:(k + 1) * NS], rws[0][k * NS:(k + 1) * NS],
             np.ones((NS, 1), np.float32)], axis=1)
        xg2 = np.concatenate(
            [feats[1][k * NS:(k + 1) * NS], rws[1][k * NS:(k + 1) * NS],
             np.ones((NS, 1), np.float32)], axis=1)
        xin = np.concatenate([xg1, xg2], axis=0).T.copy()
        in_maps.append({
            "xin": np.ascontiguousarray(xin),
            "wpre": wpre_aug, "wedge": wedge_aug,
            "efT": efT,
            "oh_d": oh,
            "ohd_d": ohd,
            "ohr_d": ohr,
            "srcov": _idx_sb(srco),
            "gw1": gw1, "gw2": gw2, "gb1t": gb1t, "gb2t": gb2t,
            "wo1": wo1, "wo2": wo2, "bo1t": bo1t, "bo2t": bo2t,
        })

    if owc not in _prog_cache:
        _prog_cache[owc] = _build_program(owc)
    nc = _prog_cache[owc]
    res = run_bass_kernel_spmd(nc, in_maps, list(range(8)), **_run_kwargs)
    global _last_result
    _last_result = res
    return np.vstack([np.asarray(res.results[k]["out"]) for k in range(8)])


_run_kwargs = {}
_last_result = None
